# revision 26
# baseline (speedup 1.0000x reference)
"""Expert-parallel MoE routing kernel for Trainium2 (8 NeuronCores).

Problem: top-k(=2) softmax-gated MoE FFN (relu), followed by
log_softmax(sum(moe_out, axis=-1)) over the sequence dim.

Key algebraic observation: the graded output is
    log_softmax_S( sum_d moe_out[t, d] )
and
    sum_d moe_out[t, :] = sum_e combine[t,e] * (relu(x_t @ W1_e + b1_e) @ rowsum(W2_e) + sum(b2_e))
so the second expert matmul collapses to a matvec against rowsum(W2_e),
which is folded on the host at dispatch time (weight preprocessing),
along with a bf16 cast of the matmul operands (x, W1, rowsum(W2)) --
measured ~3e-3 end-to-end rel err, well under the 2e-2 gate.

Sharding (per the expert-parallel hint): core e owns expert e's weights.
The host computes the (tiny) gate/top-k routing to build the dispatch
(it must, to construct the per-core input shards), gathers each expert's
tokens, and the device does the entire FFN including gate-value scaling.
Host then scatter-adds the per-(token,expert) scalars and applies the
final log_softmax on the [B, S] result.

Device-side structure (driven by trace analysis):
 - All large inputs are packed on the host into contiguous layouts and
   loaded by ~15 dma_starts, fine-grained (per kd-pair) at the front and
   ordered exactly by consumption: the DMA path ramps from ~120 GB/s to
   ~400 GB/s over its first ~6us, so the leading pieces are small and the
   first matmul only needs ~0.4MB.
 - Token capacity C is padded to 8 (not 128, i.e. 552 not 640 for the
   realized routing); C splits into <=512-col chunks (PSUM bank limit);
   chunk-major processing so the PE starts on chunk 0 early.
 - The PE HAM clock gate needs ~3.4us of sustained matmul activity to
   lift the array from 1.2 to 2.4 GHz, and DMA-wait gaps >~1.5us drop it
   back: dummy warmup matmuls run while the first loads are in flight,
   and small filler matmuls bridge the known DMA-ramp stalls inside the
   first wave. Steady-state is then ~118ns per 276-col matmul (streaming
   limit) for all 288 real matmuls.
 - relu+bias is split between the Scalar engine (activation) and the
   Vector engine (tensor_scalar add+max), and column-split on the final
   wave to shorten the tail-critical path.
 - The h@rowsum(w2) matvec is col-tiled 4x (tile_position strips at
   partition rows 0/32/64/96, DVE-combined off the critical path) for all
   but the last chunk; the last chunk keeps the serial accumulation so
   its z needs a single DVE multiply after the final matvec.
 - z = g*(h@v) + g*sum(b2): the bias is folded into a host-precomputed
   g*b2s row (skipped entirely when b2 is all-zero).
"""

import numpy as np

N_CORES = 8
P = 128
GRP = 4  # m-tiles per w1 group (one PSUM wave)


def _round_up(v, m):
    return ((v + m - 1) // m) * m


def _chunks(C):
    # even pieces of <=512 (PSUM bank holds 512 fp32); C is a multiple of 8
    n = -(-C // 512)
    base, extra = divmod(C // 8, n)
    out = []
    off = 0
    for i in range(n):
        ln = (base + (1 if i < extra else 0)) * 8
        out.append((off, ln))
        off += ln
    return out


_BUILD_CACHE = {}


def _build_program(D, H, C, b2z):
    """Trace + compile the single-core program (SPMD across 8 cores).

    Per-core inputs (packed on host):
      xtg [P, KD*C]   bf16  gathered tokens^T, chunk-major then kd-major:
                            col [ci_base + kd*ln + j] = x[token off+j, kd*128+p]
      w1p [P, MH*D]   bf16  w1 group-major: col [g*(KD*GRP*P) + kd*(GRP*P)
                            + mm*P + j] = w1[kd*128+p, (g*GRP+mm)*128 + j]
      sm  [P, 2*MH]   f32   cols 0:MH = b1t (b1 m-tile per col), MH:2*MH = v
                            (rowsum(w2) m-tile per col)
      g2  [1, 2*C]    f32   cols 0:C = gate values per slot,
                            cols C:2C = g*sum(b2)
    Output:
      z [1, C] f32 = g * (relu(x @ w1 + b1) @ rowsum(w2) + sum(b2))
    """
    key = (D, H, C, b2z)
    if key in _BUILD_CACHE:
        return _BUILD_CACHE[key]

    import concourse.tile as tile
    from concourse import bacc, mybir

    f32 = mybir.dt.float32
    bf16 = mybir.dt.bfloat16
    KD = D // P  # k-tiles over D
    MH = H // P  # m-tiles over H
    NG = MH // GRP  # w1 groups
    GCOL = KD * GRP * P  # w1p columns per group
    chunks = _chunks(C)
    NCH = len(chunks)
    cbase = [KD * off for off, ln in chunks]  # xtg col base per chunk

    nc = bacc.Bacc("TRN2", target_bir_lowering=False, debug=False)
    xtg_d = nc.dram_tensor("xtg", [P, KD * C], bf16, kind="ExternalInput").ap()
    w1_d = nc.dram_tensor("w1p", [P, MH * D], bf16, kind="ExternalInput").ap()
    sm_d = nc.dram_tensor("sm", [P, 2 * MH], f32, kind="ExternalInput").ap()
    g2_d = nc.dram_tensor("g2", [1, 2 * C], f32, kind="ExternalInput").ap()
    z_d = nc.dram_tensor("z", [1, C], f32, kind="ExternalOutput").ap()

    with tile.TileContext(nc) as tc:
        with (
            tc.tile_pool(name="persist", bufs=1) as persist,
            tc.tile_pool(name="psum_h", bufs=8 - NCH, space="PSUM") as psum_h,
            tc.tile_pool(name="psum_z", bufs=NCH, space="PSUM") as psum_z,
        ):
            # --- PE warmup: the HAM clock gate needs ~3.4us of sustained
            # matmul activity to lift the PE from 1.2 to 2.4 GHz. Dummy
            # matmuls on memset tiles run while the input DMAs are in
            # flight, so the real matmul stream starts at full clock. ---
            warm_w = persist.tile([P, 1], bf16, name="warm_w")
            warm_x = persist.tile([P, 512], bf16, name="warm_x")
            nc.vector.memset(warm_w[:], 0)
            nc.vector.memset(warm_x[:], 0)
            warm_ps = psum_z.tile([1, 512], f32, tag="psz", name="warm_ps")

            def warm_mm(n, cols=512):
                for _ in range(n):
                    nc.tensor.matmul(
                        warm_ps[:, :cols], warm_w[:], warm_x[:, :cols],
                        start=True, stop=True, skip_group_check=True,
                    )

            warm_mm(10)

            # --- loads, split across both HWDGE rings (scalar carries the
            # tokens + small tensors, sync carries the w1 stream), with the
            # first compute dependencies (kd 0-1 slices) leading ---
            sm_sb = persist.tile([P, 2 * MH], f32)
            g2_sb = persist.tile([1, 2 * C], f32)
            xtg_sb = persist.tile([P, KD * C], bf16)
            w1_sb = persist.tile([P, MH * D], bf16)

            KSPL = 2  # kd prefix that gates the first matmul

            def xtg_load(ci, k0, k1):
                off, ln = chunks[ci]
                sl = slice(cbase[ci] + k0 * ln, cbase[ci] + k1 * ln)
                nc.scalar.dma_start(out=xtg_sb[:, sl], in_=xtg_d[:, sl])

            def w1_load(g, k0, k1):
                sl = slice(g * GCOL + k0 * GRP * P, g * GCOL + k1 * GRP * P)
                nc.sync.dma_start(out=w1_sb[:, sl], in_=w1_d[:, sl])

            # All critical loads go on the sync ring in exact consumption
            # order, fine-grained (per-kd) at the front so each completion
            # semaphore releases the next few matmuls: the DMA path ramps
            # from ~120 GB/s to full rate over its first ~6us, and only the
            # sync ring starts moving data quickly (~1.4us after issue; the
            # scalar ring takes ~4us for its first byte, so it gets the
            # tensors that are needed late).
            def w1_load_sync(g, k0, k1):
                sl = slice(g * GCOL + k0 * GRP * P, g * GCOL + k1 * GRP * P)
                nc.sync.dma_start(out=w1_sb[:, sl], in_=w1_d[:, sl])

            def xtg_load_sync(ci, k0, k1):
                off, ln = chunks[ci]
                sl = slice(cbase[ci] + k0 * ln, cbase[ci] + k1 * ln)
                nc.sync.dma_start(out=xtg_sb[:, sl], in_=xtg_d[:, sl])

            xtg_load_sync(0, 0, 1)
            w1_load_sync(0, 0, 1)
            w1_load_sync(0, 1, 2)
            xtg_load_sync(0, 1, 2)
            nc.scalar.dma_start(out=sm_sb[:], in_=sm_d[:])
            w1_load_sync(0, 2, 4)
            xtg_load_sync(0, 2, 4)
            nc.scalar.dma_start(out=g2_sb[:], in_=g2_d[:])
            w1_load_sync(0, 4, 6)
            xtg_load_sync(0, 4, 6)
            w1_load_sync(0, 6, KD)
            xtg_load_sync(0, 6, KD)
            # w1 group 1 is needed right after wave 0: issue it as two
            # halves so its first kd-tiles complete (and release matmuls)
            # without waiting for the whole group
            if NG > 1:
                w1_load_sync(1, 0, KD // 2)
                w1_load(1, KD // 2, KD)
            for ci in range(1, NCH):
                xtg_load(ci, 0, KD)
            for g in range(2, NG):
                w1_load(g, 0, KD)

            # bf16 cast of rowsum(w2) (DVE, ~0.1us)
            v_sb = persist.tile([P, MH], bf16)
            with nc.allow_low_precision(reason="matmul operand is bf16 anyway"):
                nc.vector.tensor_copy(out=v_sb[:], in_=sm_sb[:, MH : 2 * MH])

            # hT per chunk: [128, MH * chunk_len], slice m holds h^T m-tile
            ht_tiles = [
                persist.tile([P, MH * ln], bf16, tag=f"ht{ci}", name=f"ht{ci}")
                for ci, (off, ln) in enumerate(chunks)
            ]
            z_sb = persist.tile([1, C], f32)
            # matvec accumulators: 4 col-tiled PE strips (partition rows
            # 0/32/64/96, DVE-combined later) for all but the last chunk;
            # the last chunk keeps the serial single-row accumulation so
            # its z needs only one DVE op after the final matvec
            pzs = [
                psum_z.tile([128 if ci < NCH - 1 else 1, ln], f32,
                            tag="psz", name="psz")
                for ci, (off, ln) in enumerate(chunks)
            ]
            zt_sb = persist.tile([1, C], f32, name="zt")

            def mm1_step(ci, g):
                off, ln = chunks[ci]
                pss = [
                    psum_h.tile([P, ln], f32, tag="psh", name="psh")
                    for _ in range(GRP)
                ]
                first = ci == 0 and g == 0
                for kd in range(KD):
                    base = g * GCOL + kd * (GRP * P)
                    for mm in range(GRP):
                        nc.tensor.matmul(
                            pss[mm][:],
                            w1_sb[:, base + mm * P : base + (mm + 1) * P],
                            xtg_sb[:, cbase[ci] + kd * ln : cbase[ci] + (kd + 1) * ln],
                            start=(kd == 0),
                            stop=(kd == KD - 1),
                            skip_group_check=True,
                        )
                    # keep the PE busy across the DMA-ramp stalls of the
                    # first wave so the HAM clock gate lifts early and stays
                    # lifted (a broken busy-streak leaves the PE at 1.2 GHz)
                    if first and kd == 1:
                        warm_mm(8, 256)
                    elif first and kd == 3:
                        warm_mm(4, 256)
                    elif first and kd == 5:
                        warm_mm(3, 256)
                # relu+bias, split scalar/vector to halve drain latency;
                # on the final wave split each tile column-wise too, so the
                # last matvec (tail-critical) starts sooner
                last_wave = (ci == NCH - 1) and (g == NG - 1)
                for mm in range(GRP):
                    m = g * GRP + mm
                    halves = (
                        [(0, ln // 2), (ln // 2, ln)] if last_wave else [(0, ln)]
                    )
                    for hi, (c0, c1) in enumerate(halves):
                        dst = ht_tiles[ci][:, m * ln + c0 : m * ln + c1]
                        if (mm + hi) % 2 == 0:
                            nc.scalar.activation(
                                dst,
                                pss[mm][:, c0:c1],
                                mybir.ActivationFunctionType.Relu,
                                bias=sm_sb[:, m : m + 1],
                            )
                        else:
                            with nc.allow_low_precision(reason="h is bf16 anyway"):
                                nc.vector.tensor_scalar(
                                    out=dst,
                                    in0=pss[mm][:, c0:c1],
                                    scalar1=sm_sb[:, m : m + 1],
                                    scalar2=0.0,
                                    op0=mybir.AluOpType.add,
                                    op1=mybir.AluOpType.max,
                                )

            def matvec_step(ci, g):
                off, ln = chunks[ci]
                for mm in range(GRP):
                    m = g * GRP + mm
                    if ci < NCH - 1:
                        # 4 concurrent matvecs in 32-col PE strips: strip mm
                        # accumulates m = g*GRP+mm over g into row 32*mm
                        nc.tensor.matmul(
                            pzs[ci][32 * mm : 32 * mm + 1, :],
                            v_sb[:, m : m + 1],
                            ht_tiles[ci][:, m * ln : (m + 1) * ln],
                            start=(g == 0),
                            stop=(g == NG - 1),
                            skip_group_check=True,
                            tile_position=(0, 32 * mm),
                        )
                    else:
                        nc.tensor.matmul(
                            pzs[ci][:],
                            v_sb[:, m : m + 1],
                            ht_tiles[ci][:, m * ln : (m + 1) * ln],
                            start=(m == 0),
                            stop=(m == MH - 1),
                            skip_group_check=True,
                        )

            def z_step(ci):
                off, ln = chunks[ci]
                dst = z_sb[:, off : off + ln]
                tmp = zt_sb[:, off : off + ln]
                pz = pzs[ci]
                if ci < NCH - 1:
                    # combine the 4 strip rows (DVE reads at most one PSUM
                    # operand per op); this chain hides under later waves
                    nc.vector.tensor_copy(out=tmp, in_=pz[32:33, :])
                    nc.vector.tensor_add(tmp, pz[0:1, :], tmp)
                    nc.vector.tensor_add(tmp, pz[64:65, :], tmp)
                    nc.vector.tensor_add(tmp, pz[96:97, :], tmp)
                    nc.vector.tensor_mul(dst, tmp, g2_sb[:, off : off + ln])
                else:
                    nc.vector.tensor_mul(dst, pz[0:1, :], g2_sb[:, off : off + ln])
                if not b2z:
                    nc.vector.tensor_add(
                        dst, dst, g2_sb[:, C + off : C + off + ln]
                    )
                nc.sync.dma_start(out=z_d[:, off : off + ln], in_=dst)

            # mm1 wave i overlaps matvec wave i-1 so the PE never waits on
            # the relu drain of the group it just produced
            steps = [(ci, g) for ci in range(NCH) for g in range(NG)]
            for i, (ci, g) in enumerate(steps):
                if i == 1:
                    warm_mm(3, 256)
                mm1_step(ci, g)
                if i > 0:
                    matvec_step(*steps[i - 1])
                    if steps[i - 1][1] == NG - 1:
                        z_step(steps[i - 1][0])
            matvec_step(*steps[-1])
            z_step(steps[-1][0])

    nc.compile()
    _BUILD_CACHE[key] = nc
    return nc


def kernel(x, wg, w1, b1, w2, b2, k):
    import ml_dtypes
    from concourse.bass_utils import run_bass_kernel_spmd

    bf16 = ml_dtypes.bfloat16

    x = np.asarray(x)
    wg = np.asarray(wg)
    w1 = np.asarray(w1)
    b1 = np.asarray(b1)
    w2 = np.asarray(w2)
    b2 = np.asarray(b2)
    k = int(k)

    B, S, D = x.shape
    E = wg.shape[1]
    H = w1.shape[2]
    T = B * S
    KD = D // P
    MH = H // P
    assert E == N_CORES, f"expert-parallel layout assumes E == 8, got {E}"
    assert D % P == 0 and H % (P * GRP) == 0, (D, H)

    xf = np.ascontiguousarray(x.reshape(T, D), dtype=np.float32)

    # --- gate + top-k routing (host; needed to build the dispatch shards) ---
    logits = xf @ wg.astype(np.float32)
    logits -= logits.max(axis=1, keepdims=True)
    np.exp(logits, out=logits)
    scores = logits / logits.sum(axis=1, keepdims=True)
    if k >= E:
        topi = np.broadcast_to(np.arange(E, dtype=np.int64), (T, E))
    else:
        topi = np.argpartition(-scores, k, axis=1)[:, :k]
    rows = np.arange(T)[:, None]
    topv = scores[rows, topi]

    # per-expert token lists
    idx_e = []
    val_e = []
    for e in range(E):
        tmask, kpos = np.nonzero(topi == e)
        idx_e.append(tmask)
        val_e.append(topv[tmask, kpos].astype(np.float32))
    max_cnt = max(len(i) for i in idx_e)
    C = max(256, _round_up(max_cnt, 8))
    chunks = _chunks(C)

    b2z = not np.any(b2)
    nc = _build_program(D, H, C, b2z)

    xf16 = xf.astype(bf16)
    w116 = w1.astype(bf16)
    in_maps = []
    for e in range(E):
        n_e = len(idx_e[e])
        # tokens^T, chunk-major then kd-major: [KD, P, C] -> per-chunk
        # [P, KD, ln] blocks concatenated along columns
        xg = np.zeros((KD, P, C), dtype=bf16)
        xg.reshape(D, C)[:, :n_e] = xf16[idx_e[e]].T
        xtg = np.concatenate(
            [
                np.ascontiguousarray(xg[:, :, off : off + ln].transpose(1, 0, 2)).reshape(
                    P, KD * ln
                )
                for off, ln in chunks
            ],
            axis=1,
        )
        # w1 group-major: [KD, P, NG, GRP*P] -> [P, NG, KD, GRP*P]
        w1g = (
            w116[e]
            .reshape(KD, P, MH // GRP, GRP * P)
            .transpose(1, 2, 0, 3)
            .reshape(P, MH * D)
        )
        # fold w2/b2: only rowsum(w2) and sum(b2) are needed downstream
        v = w2[e].astype(np.float32).sum(axis=1)  # [H]
        sm = np.empty((P, 2 * MH), dtype=np.float32)
        sm[:, :MH] = b1[e].astype(np.float32).reshape(MH, P).T
        sm[:, MH:] = v.reshape(MH, P).T
        b2s = float(b2[e].astype(np.float64).sum())
        g2 = np.zeros((1, 2 * C), dtype=np.float32)
        g2[0, :n_e] = val_e[e]
        g2[0, C : C + n_e] = val_e[e] * b2s
        in_maps.append(
            {
                "xtg": xtg,
                "w1p": np.ascontiguousarray(w1g),
                "sm": sm,
                "g2": g2,
            }
        )

    res = run_bass_kernel_spmd(nc, in_maps, core_ids=list(range(N_CORES)))

    # --- combine: scatter-add per-(token, expert) scalars, then log_softmax ---
    s = np.zeros(T, dtype=np.float32)
    for e in range(E):
        n_e = len(idx_e[e])
        if n_e:
            s[idx_e[e]] += res.results[e]["z"][0, :n_e]

    sm_ = s.reshape(B, S)
    sm_ = sm_ - sm_.max(axis=1, keepdims=True)
    out = sm_ - np.log(np.exp(sm_).sum(axis=1, keepdims=True))
    return out.astype(np.float32)


# revision 27
# speedup vs baseline: 1.0309x; 1.0309x over previous
"""Expert-parallel MoE routing kernel for Trainium2 (8 NeuronCores).

Problem: top-k(=2) softmax-gated MoE FFN (relu), followed by
log_softmax(sum(moe_out, axis=-1)) over the sequence dim.

Key algebraic observation: the graded output is
    log_softmax_S( sum_d moe_out[t, d] )
and
    sum_d moe_out[t, :] = sum_e combine[t,e] * (relu(x_t @ W1_e + b1_e) @ rowsum(W2_e) + sum(b2_e))
so the second expert matmul collapses to a matvec against rowsum(W2_e),
which is folded on the host at dispatch time (weight preprocessing),
along with a bf16 cast of the matmul operands (x, W1, rowsum(W2)) --
measured ~3e-3 end-to-end rel err, well under the 2e-2 gate.

Sharding (per the expert-parallel hint): core e owns expert e's weights.
The host computes the (tiny) gate/top-k routing to build the dispatch
(it must, to construct the per-core input shards), gathers each expert's
tokens, and the device does the entire FFN including gate-value scaling.
Host then scatter-adds the per-(token,expert) scalars and applies the
final log_softmax on the [B, S] result.

Device-side structure (driven by trace analysis):
 - All large inputs are packed on the host into contiguous layouts and
   loaded by ~15 dma_starts, fine-grained (per kd-pair) at the front and
   ordered exactly by consumption: the DMA path ramps from ~120 GB/s to
   ~400 GB/s over its first ~6us, so the leading pieces are small and the
   first matmul only needs ~0.4MB.
 - Token capacity C is padded to 8 (not 128, i.e. 552 not 640 for the
   realized routing); C splits into <=512-col chunks (PSUM bank limit);
   chunk-major processing so the PE starts on chunk 0 early.
 - The PE HAM clock gate needs ~3.4us of sustained matmul activity to
   lift the array from 1.2 to 2.4 GHz, and DMA-wait gaps >~1.5us drop it
   back: dummy warmup matmuls run while the first loads are in flight,
   and small filler matmuls bridge the known DMA-ramp stalls inside the
   first wave. Steady-state is then ~118ns per 276-col matmul (streaming
   limit) for all 288 real matmuls.
 - relu+bias is split between the Scalar engine (activation) and the
   Vector engine (tensor_scalar add+max), and column-split on the final
   wave to shorten the tail-critical path.
 - The h@rowsum(w2) matvec is col-tiled 4x (tile_position strips at
   partition rows 0/32/64/96, DVE-combined off the critical path) for all
   but the last chunk; the last chunk keeps the serial accumulation so
   its z needs a single DVE multiply after the final matvec.
 - z = g*(h@v) + g*sum(b2): the bias is folded into a host-precomputed
   g*b2s row (skipped entirely when b2 is all-zero).
"""

import numpy as np

N_CORES = 8
P = 128
GRP = 4  # m-tiles per w1 group (one PSUM wave)


def _round_up(v, m):
    return ((v + m - 1) // m) * m


def _chunks(C):
    # even pieces of <=512 (PSUM bank holds 512 fp32); C is a multiple of 8
    n = -(-C // 512)
    base, extra = divmod(C // 8, n)
    out = []
    off = 0
    for i in range(n):
        ln = (base + (1 if i < extra else 0)) * 8
        out.append((off, ln))
        off += ln
    return out


_BUILD_CACHE = {}


def _build_program(D, H, C, b2z):
    """Trace + compile the single-core program (SPMD across 8 cores).

    Per-core inputs (packed on host):
      xtg [P, KD*C]   bf16  gathered tokens^T, chunk-major then kd-major:
                            col [ci_base + kd*ln + j] = x[token off+j, kd*128+p]
      w1p [P, MH*D]   bf16  w1 group-major: col [g*(KD*GRP*P) + kd*(GRP*P)
                            + mm*P + j] = w1[kd*128+p, (g*GRP+mm)*128 + j]
      sm  [P, 2*MH]   f32   cols 0:MH = b1t (b1 m-tile per col), MH:2*MH = v
                            (rowsum(w2) m-tile per col)
      g2  [1, 2*C]    f32   cols 0:C = gate values per slot,
                            cols C:2C = g*sum(b2)
    Output:
      z [1, C] f32 = g * (relu(x @ w1 + b1) @ rowsum(w2) + sum(b2))
    """
    key = (D, H, C, b2z)
    if key in _BUILD_CACHE:
        return _BUILD_CACHE[key]

    import concourse.tile as tile
    from concourse import bacc, mybir

    f32 = mybir.dt.float32
    bf16 = mybir.dt.bfloat16
    KD = D // P  # k-tiles over D
    MH = H // P  # m-tiles over H
    NG = MH // GRP  # w1 groups
    GCOL = KD * GRP * P  # w1p columns per group
    chunks = _chunks(C)
    NCH = len(chunks)
    cbase = [KD * off for off, ln in chunks]  # xtg col base per chunk

    nc = bacc.Bacc("TRN2", target_bir_lowering=False, debug=False)
    xtg_d = nc.dram_tensor("xtg", [P, KD * C], bf16, kind="ExternalInput").ap()
    w1_d = nc.dram_tensor("w1p", [P, MH * D], bf16, kind="ExternalInput").ap()
    sm_d = nc.dram_tensor("sm", [P, 2 * MH], f32, kind="ExternalInput").ap()
    g2_d = nc.dram_tensor("g2", [1, 2 * C], f32, kind="ExternalInput").ap()
    z_d = nc.dram_tensor("z", [1, C], f32, kind="ExternalOutput").ap()

    with tile.TileContext(nc) as tc:
        with (
            tc.tile_pool(name="persist", bufs=1) as persist,
            tc.tile_pool(name="psum_h", bufs=8 - NCH, space="PSUM") as psum_h,
            tc.tile_pool(name="psum_z", bufs=NCH, space="PSUM") as psum_z,
        ):
            # --- PE warmup: the HAM clock gate needs ~3.4us of sustained
            # matmul activity to lift the PE from 1.2 to 2.4 GHz. Dummy
            # matmuls on memset tiles run while the input DMAs are in
            # flight, so the real matmul stream starts at full clock. ---
            warm_w = persist.tile([P, 1], bf16, name="warm_w")
            warm_x = persist.tile([P, 512], bf16, name="warm_x")
            nc.vector.memset(warm_w[:], 0)
            nc.vector.memset(warm_x[:], 0)
            warm_ps = psum_z.tile([1, 512], f32, tag="psz", name="warm_ps")

            def warm_mm(n, cols=512):
                for _ in range(n):
                    nc.tensor.matmul(
                        warm_ps[:, :cols], warm_w[:], warm_x[:, :cols],
                        start=True, stop=True, skip_group_check=True,
                    )

            warm_mm(10)

            # --- loads, split across both HWDGE rings (scalar carries the
            # tokens + small tensors, sync carries the w1 stream), with the
            # first compute dependencies (kd 0-1 slices) leading ---
            sm_sb = persist.tile([P, 2 * MH], f32)
            g2_sb = persist.tile([1, 2 * C], f32)
            xtg_sb = persist.tile([P, KD * C], bf16)
            w1_sb = persist.tile([P, MH * D], bf16)

            KSPL = 2  # kd prefix that gates the first matmul

            def xtg_load(ci, k0, k1):
                off, ln = chunks[ci]
                sl = slice(cbase[ci] + k0 * ln, cbase[ci] + k1 * ln)
                nc.scalar.dma_start(out=xtg_sb[:, sl], in_=xtg_d[:, sl])

            def w1_load(g, k0, k1):
                sl = slice(g * GCOL + k0 * GRP * P, g * GCOL + k1 * GRP * P)
                nc.sync.dma_start(out=w1_sb[:, sl], in_=w1_d[:, sl])

            # All critical loads go on the sync ring in exact consumption
            # order, fine-grained (per-kd) at the front so each completion
            # semaphore releases the next few matmuls: the DMA path ramps
            # from ~120 GB/s to full rate over its first ~6us, and only the
            # sync ring starts moving data quickly (~1.4us after issue; the
            # scalar ring takes ~4us for its first byte, so it gets the
            # tensors that are needed late).
            def w1_load_sync(g, k0, k1):
                sl = slice(g * GCOL + k0 * GRP * P, g * GCOL + k1 * GRP * P)
                nc.sync.dma_start(out=w1_sb[:, sl], in_=w1_d[:, sl])

            def xtg_load_sync(ci, k0, k1):
                off, ln = chunks[ci]
                sl = slice(cbase[ci] + k0 * ln, cbase[ci] + k1 * ln)
                nc.sync.dma_start(out=xtg_sb[:, sl], in_=xtg_d[:, sl])

            xtg_load_sync(0, 0, 1)
            w1_load_sync(0, 0, 1)
            w1_load_sync(0, 1, 2)
            xtg_load_sync(0, 1, 2)
            nc.scalar.dma_start(out=sm_sb[:], in_=sm_d[:])
            w1_load_sync(0, 2, 4)
            xtg_load_sync(0, 2, 4)
            nc.scalar.dma_start(out=g2_sb[:], in_=g2_d[:])
            w1_load_sync(0, 4, 6)
            xtg_load_sync(0, 4, 6)
            w1_load_sync(0, 6, KD)
            xtg_load_sync(0, 6, KD)
            # w1 group 1 is needed right after wave 0: issue it as two
            # halves so its first kd-tiles complete (and release matmuls)
            # without waiting for the whole group
            if NG > 1:
                w1_load_sync(1, 0, KD // 2)
                w1_load(1, KD // 2, KD)
            # chunk-B tokens, consumed only in the second half of the
            # kernel, are split: half on the (otherwise idle) scalar ring,
            # half last on sync -- a full xtgB on scalar competes with the
            # critical w1 stream for HBM during the ramp window, while a
            # full xtgB on sync pushes the w1g2 deadline too tight
            for ci in range(1, NCH):
                xtg_load(ci, 0, KD // 2)
            for g in range(2, NG):
                w1_load(g, 0, KD)
            for ci in range(1, NCH):
                xtg_load_sync(ci, KD // 2, KD)

            # bf16 cast of rowsum(w2) (DVE, ~0.1us)
            v_sb = persist.tile([P, MH], bf16)
            with nc.allow_low_precision(reason="matmul operand is bf16 anyway"):
                nc.vector.tensor_copy(out=v_sb[:], in_=sm_sb[:, MH : 2 * MH])

            # hT per chunk: [128, MH * chunk_len], slice m holds h^T m-tile
            ht_tiles = [
                persist.tile([P, MH * ln], bf16, tag=f"ht{ci}", name=f"ht{ci}")
                for ci, (off, ln) in enumerate(chunks)
            ]
            z_sb = persist.tile([1, C], f32)
            # matvec accumulators: 4 col-tiled PE strips (partition rows
            # 0/32/64/96, DVE-combined later) for all but the last chunk;
            # the last chunk keeps the serial single-row accumulation so
            # its z needs only one DVE op after the final matvec
            pzs = [
                psum_z.tile([128 if ci < NCH - 1 else 1, ln], f32,
                            tag="psz", name="psz")
                for ci, (off, ln) in enumerate(chunks)
            ]
            zt_sb = persist.tile([1, C], f32, name="zt")

            def mm1_step(ci, g):
                off, ln = chunks[ci]
                pss = [
                    psum_h.tile([P, ln], f32, tag="psh", name="psh")
                    for _ in range(GRP)
                ]
                first = ci == 0 and g == 0
                for kd in range(KD):
                    base = g * GCOL + kd * (GRP * P)
                    for mm in range(GRP):
                        nc.tensor.matmul(
                            pss[mm][:],
                            w1_sb[:, base + mm * P : base + (mm + 1) * P],
                            xtg_sb[:, cbase[ci] + kd * ln : cbase[ci] + (kd + 1) * ln],
                            start=(kd == 0),
                            stop=(kd == KD - 1),
                            skip_group_check=True,
                        )
                    # keep the PE busy across the DMA-ramp stalls of the
                    # first wave so the HAM clock gate lifts early and stays
                    # lifted (a broken busy-streak leaves the PE at 1.2 GHz)
                    if first and kd == 1:
                        warm_mm(8, 256)
                    elif first and kd == 3:
                        warm_mm(4, 256)
                    elif first and kd == 5:
                        warm_mm(3, 256)
                # relu+bias, split scalar/vector to halve drain latency;
                # on the final wave split each tile column-wise too, so the
                # last matvec (tail-critical) starts sooner
                last_wave = (ci == NCH - 1) and (g == NG - 1)
                for mm in range(GRP):
                    m = g * GRP + mm
                    halves = (
                        [(0, ln // 2), (ln // 2, ln)] if last_wave else [(0, ln)]
                    )
                    for hi, (c0, c1) in enumerate(halves):
                        dst = ht_tiles[ci][:, m * ln + c0 : m * ln + c1]
                        if (mm + hi) % 2 == 0:
                            nc.scalar.activation(
                                dst,
                                pss[mm][:, c0:c1],
                                mybir.ActivationFunctionType.Relu,
                                bias=sm_sb[:, m : m + 1],
                            )
                        else:
                            with nc.allow_low_precision(reason="h is bf16 anyway"):
                                nc.vector.tensor_scalar(
                                    out=dst,
                                    in0=pss[mm][:, c0:c1],
                                    scalar1=sm_sb[:, m : m + 1],
                                    scalar2=0.0,
                                    op0=mybir.AluOpType.add,
                                    op1=mybir.AluOpType.max,
                                )

            def matvec_step(ci, g):
                off, ln = chunks[ci]
                for mm in range(GRP):
                    m = g * GRP + mm
                    if ci < NCH - 1:
                        # 4 concurrent matvecs in 32-col PE strips: strip mm
                        # accumulates m = g*GRP+mm over g into row 32*mm
                        nc.tensor.matmul(
                            pzs[ci][32 * mm : 32 * mm + 1, :],
                            v_sb[:, m : m + 1],
                            ht_tiles[ci][:, m * ln : (m + 1) * ln],
                            start=(g == 0),
                            stop=(g == NG - 1),
                            skip_group_check=True,
                            tile_position=(0, 32 * mm),
                        )
                    else:
                        nc.tensor.matmul(
                            pzs[ci][:],
                            v_sb[:, m : m + 1],
                            ht_tiles[ci][:, m * ln : (m + 1) * ln],
                            start=(m == 0),
                            stop=(m == MH - 1),
                            skip_group_check=True,
                        )

            def z_step(ci):
                off, ln = chunks[ci]
                dst = z_sb[:, off : off + ln]
                tmp = zt_sb[:, off : off + ln]
                pz = pzs[ci]
                if ci < NCH - 1:
                    # combine the 4 strip rows (DVE reads at most one PSUM
                    # operand per op); this chain hides under later waves
                    nc.vector.tensor_copy(out=tmp, in_=pz[32:33, :])
                    nc.vector.tensor_add(tmp, pz[0:1, :], tmp)
                    nc.vector.tensor_add(tmp, pz[64:65, :], tmp)
                    nc.vector.tensor_add(tmp, pz[96:97, :], tmp)
                    nc.vector.tensor_mul(dst, tmp, g2_sb[:, off : off + ln])
                else:
                    nc.vector.tensor_mul(dst, pz[0:1, :], g2_sb[:, off : off + ln])
                if not b2z:
                    nc.vector.tensor_add(
                        dst, dst, g2_sb[:, C + off : C + off + ln]
                    )
                nc.sync.dma_start(out=z_d[:, off : off + ln], in_=dst)

            # mm1 wave i overlaps matvec wave i-1 so the PE never waits on
            # the relu drain of the group it just produced
            steps = [(ci, g) for ci in range(NCH) for g in range(NG)]
            for i, (ci, g) in enumerate(steps):
                if i == 1:
                    warm_mm(3, 256)
                mm1_step(ci, g)
                if i > 0:
                    matvec_step(*steps[i - 1])
                    if steps[i - 1][1] == NG - 1:
                        z_step(steps[i - 1][0])
            matvec_step(*steps[-1])
            z_step(steps[-1][0])

    nc.compile()
    _BUILD_CACHE[key] = nc
    return nc


def kernel(x, wg, w1, b1, w2, b2, k):
    import ml_dtypes
    from concourse.bass_utils import run_bass_kernel_spmd

    bf16 = ml_dtypes.bfloat16

    x = np.asarray(x)
    wg = np.asarray(wg)
    w1 = np.asarray(w1)
    b1 = np.asarray(b1)
    w2 = np.asarray(w2)
    b2 = np.asarray(b2)
    k = int(k)

    B, S, D = x.shape
    E = wg.shape[1]
    H = w1.shape[2]
    T = B * S
    KD = D // P
    MH = H // P
    assert E == N_CORES, f"expert-parallel layout assumes E == 8, got {E}"
    assert D % P == 0 and H % (P * GRP) == 0, (D, H)

    xf = np.ascontiguousarray(x.reshape(T, D), dtype=np.float32)

    # --- gate + top-k routing (host; needed to build the dispatch shards) ---
    logits = xf @ wg.astype(np.float32)
    logits -= logits.max(axis=1, keepdims=True)
    np.exp(logits, out=logits)
    scores = logits / logits.sum(axis=1, keepdims=True)
    if k >= E:
        topi = np.broadcast_to(np.arange(E, dtype=np.int64), (T, E))
    else:
        topi = np.argpartition(-scores, k, axis=1)[:, :k]
    rows = np.arange(T)[:, None]
    topv = scores[rows, topi]

    # per-expert token lists
    idx_e = []
    val_e = []
    for e in range(E):
        tmask, kpos = np.nonzero(topi == e)
        idx_e.append(tmask)
        val_e.append(topv[tmask, kpos].astype(np.float32))
    max_cnt = max(len(i) for i in idx_e)
    C = max(256, _round_up(max_cnt, 8))
    chunks = _chunks(C)

    b2z = not np.any(b2)
    nc = _build_program(D, H, C, b2z)

    xf16 = xf.astype(bf16)
    w116 = w1.astype(bf16)
    in_maps = []
    for e in range(E):
        n_e = len(idx_e[e])
        # tokens^T, chunk-major then kd-major: [KD, P, C] -> per-chunk
        # [P, KD, ln] blocks concatenated along columns
        xg = np.zeros((KD, P, C), dtype=bf16)
        xg.reshape(D, C)[:, :n_e] = xf16[idx_e[e]].T
        xtg = np.concatenate(
            [
                np.ascontiguousarray(xg[:, :, off : off + ln].transpose(1, 0, 2)).reshape(
                    P, KD * ln
                )
                for off, ln in chunks
            ],
            axis=1,
        )
        # w1 group-major: [KD, P, NG, GRP*P] -> [P, NG, KD, GRP*P]
        w1g = (
            w116[e]
            .reshape(KD, P, MH // GRP, GRP * P)
            .transpose(1, 2, 0, 3)
            .reshape(P, MH * D)
        )
        # fold w2/b2: only rowsum(w2) and sum(b2) are needed downstream
        v = w2[e].astype(np.float32).sum(axis=1)  # [H]
        sm = np.empty((P, 2 * MH), dtype=np.float32)
        sm[:, :MH] = b1[e].astype(np.float32).reshape(MH, P).T
        sm[:, MH:] = v.reshape(MH, P).T
        b2s = float(b2[e].astype(np.float64).sum())
        g2 = np.zeros((1, 2 * C), dtype=np.float32)
        g2[0, :n_e] = val_e[e]
        g2[0, C : C + n_e] = val_e[e] * b2s
        in_maps.append(
            {
                "xtg": xtg,
                "w1p": np.ascontiguousarray(w1g),
                "sm": sm,
                "g2": g2,
            }
        )

    res = run_bass_kernel_spmd(nc, in_maps, core_ids=list(range(N_CORES)))

    # --- combine: scatter-add per-(token, expert) scalars, then log_softmax ---
    s = np.zeros(T, dtype=np.float32)
    for e in range(E):
        n_e = len(idx_e[e])
        if n_e:
            s[idx_e[e]] += res.results[e]["z"][0, :n_e]

    sm_ = s.reshape(B, S)
    sm_ = sm_ - sm_.max(axis=1, keepdims=True)
    out = sm_ - np.log(np.exp(sm_).sum(axis=1, keepdims=True))
    return out.astype(np.float32)
